# revision 2
# baseline (speedup 1.0000x reference)
"""3-layer GAT on 8 Trainium2 NeuronCores (graph/data parallel by dst node).

v2: gather-descriptor-minimized design. Pool-engine SWDGE desc-gen (~3ns/row)
is the hard bottleneck, so:
  - Table rows are 256B (128 x bf16 h only). a_src/a_dst per edge come from
    host-built dense streams (same trick the baseline used for a_dst alone),
    so nothing but h is gathered.
  - Exact per-(block,half) gather counts (max over the 8 cores, rounded to
    16) instead of a uniform K: ~15% fewer gathered rows, ~14% fewer matmul/
    one-hot chunks. Per-block chunk counts K_b vary; SBUF tiles are sized to
    KMAX and only the first K_b chunks are touched.
  - Slots beyond the gathered count hold stale-but-finite data; their edge
    weight w = exp(prelu(-1e30)) = 0 kills any contribution. G buffers are
    memset once at kernel start so the very first blocks see finite bits.
  - Dense projections run in bf16; dense phase emits a 256B h row plus a
    separate 32B aux row (a_src/a_dst per node) that feeds the host streams.
"""

import os
import sys
import copy
import types
import numpy as np

if "/opt/trn_rl_repo" not in sys.path:
    sys.path.insert(0, "/opt/trn_rl_repo")

N, E = 50000, 800000
NEG = 0.2

NCORES = 8
BLOCKS = 49                    # per core
NPC = BLOCKS * 128             # nodes per core = 6272
NPAD = NCORES * NPC            # 50176
TROWS = 50432                  # table rows (node n -> row n), >= NPAD
SPLIT = 32768                  # lo window rows [0, SPLIT), hi [SPLIT, TROWS)
ROWF = 64                      # f32 words per table row (256B = 128 bf16)
CALL_MAX = 1024                # SWDGE ring limit per gather call


# --------------------------------------------------------------------------
# harness shims
# --------------------------------------------------------------------------
def _install_ntff_hook():
    """Register the NTFF profile hook the agent image's antenv lacks, so
    run_bass_kernel_spmd(trace=True) can report exec_time_ns."""
    try:
        import antenv
        if getattr(antenv, "axon_hooks", None) is not None:
            return True
        mod = types.ModuleType("antenv.axon_hooks")
        hook = [None]
        mod.set_axon_ntff_profile_hook = lambda h: hook.__setitem__(0, h)
        mod.get_axon_ntff_profile_hook = lambda: hook[0]
        antenv.axon_hooks = mod
        sys.modules["antenv.axon_hooks"] = mod
        from trn_agent_boot.trn_boot import _ntff_profile_via_ctypes
        mod.set_axon_ntff_profile_hook(
            _ntff_profile_via_ctypes("/opt/axon/libaxon_pjrt.so"))
        return hook[0] is not None
    except Exception:
        return False


def _split_multiwait_ctrl(nc, max_waits=1):
    """This walrus build rejects >1 semaphore wait on CTRL-class (Drain/Nop)
    instructions; split the TileContext tail drain into single-wait clones."""
    for bb in nc.main_func.blocks:
        newlist = []
        for ins in bb.instructions:
            si = ins.sync_info
            if (si is not None and si.on_wait and len(si.on_wait) > max_waits
                    and type(ins).__name__ in ("InstDrain", "InstNop")):
                waits = list(si.on_wait)
                si.on_wait = type(si.on_wait)([waits[0]])
                for i, w in enumerate(waits[1:]):
                    cl = copy.deepcopy(ins)
                    cl.name = f"{ins.name}-wsplit{i}"
                    cl.sync_info = copy.deepcopy(si)
                    cl.sync_info.on_wait = type(si.on_wait)([w])
                    cl.sync_info.on_update = type(si.on_update)([])
                    nc.register_instruction(cl, overwrite=True)
                    newlist.append(cl)
            newlist.append(ins)
        bb.instructions[:] = newlist
    return nc


# --------------------------------------------------------------------------
# host-side graph prep (static per graph, layer-independent)
# --------------------------------------------------------------------------
def _round16(n):
    return (n + 15) // 16 * 16


def _split_calls(n):
    """Split n rows (mult of 16) into even-ish calls of <= CALL_MAX rows,
    each a multiple of 16."""
    ncalls = max(1, -(-n // CALL_MAX))
    out = []
    rem = n
    for i in range(ncalls, 0, -1):
        c = _round16(-(-rem // i))
        c = min(c, rem)
        out.append(c)
        rem -= c
    assert rem == 0 and all(c <= CALL_MAX for c in out)
    return out


def _wrap_idx(idx):
    """[ni] -> [128, ni//16] int16 in dma_gather's 16-partition wrapped
    layout, replicated to all 8 GPSIMD cores."""
    ni = idx.shape[0]
    w = np.zeros((16, ni // 16), dtype=np.int16)
    w[np.arange(ni) % 16, np.arange(ni) // 16] = idx
    return np.tile(w, (8, 1))


def _prep_graph(edge_index):
    # self-loops are NOT bucketed: each block's own 128 rows arrive via one
    # dense HWDGE DMA into chunk 0 (no Pool-engine gather descriptors)
    src = np.asarray(edge_index[0], np.int64)
    dst = np.asarray(edge_index[1], np.int64)
    core = dst // NPC
    blk = (dst % NPC) // 128
    half = (src >= SPLIT).astype(np.int64)
    key = (core * BLOCKS + blk) * 2 + half
    order = np.argsort(key, kind="stable")
    ks = key[order]
    bounds = np.searchsorted(ks, np.arange(NCORES * BLOCKS * 2 + 1))
    counts = np.diff(bounds).reshape(NCORES, BLOCKS, 2)

    # static per-(block,half) gather sizes: max over cores, rounded to 16
    n16 = _round16(counts.max(axis=0))                  # [BLOCKS, 2]
    # call structure + chunk layout per block (shared by all cores);
    # chunk 0 of every block is the dense self-loop slice
    calls = []       # per block: list of (half, rows, chunk_base)
    kofs = []        # chunk offset of each block
    K_b = []
    ktot = 0
    for b in range(BLOCKS):
        kofs.append(ktot)
        cl = []
        kk = 1
        for h in (0, 1):
            for c in _split_calls(int(n16[b, h])):
                cl.append((h, c, kk))
                kk += -(-c // 128)
        calls.append(cl)
        K_b.append(kk)
        ktot += kk
    KMAX = max(K_b)

    per_core = []
    for c in range(NCORES):
        gidx_parts = []
        cnt16 = []
        dstrel = np.zeros((128, ktot), dtype=np.float32)
        srcmap = np.full((ktot, 128), -1, dtype=np.int64)
        dstmap = np.full((ktot, 128), -1, dtype=np.int64)
        for b in range(BLOCKS):
            selfn = c * NPC + b * 128 + np.arange(128, dtype=np.int64)
            srcmap[kofs[b]] = selfn
            dstmap[kofs[b]] = selfn
            dstrel[:, kofs[b]] = np.arange(128, dtype=np.float32)
            start = {}
            for h in (0, 1):
                g = (c * BLOCKS + b) * 2 + h
                es = order[bounds[g]:bounds[g + 1]]
                start[h] = [es, 0]
            for h, rows, kk in calls[b]:
                es, off = start[h]
                take = es[off:off + rows]
                start[h][1] = off + rows
                ne = len(take)
                idx = np.zeros(rows, np.int64)
                if ne:
                    s = src[take]
                    idx[:ne] = s if h == 0 else s - SPLIT
                kbase = kofs[b] + kk
                nchunk = -(-rows // 128)
                dd = np.full(nchunk * 128, -1, np.int64)
                ss = np.full(nchunk * 128, -1, np.int64)
                if ne:
                    dd[:ne] = dst[take]
                    ss[:ne] = src[take]
                dm = dd.reshape(nchunk, 128)
                sm = ss.reshape(nchunk, 128)
                dstmap[kbase:kbase + nchunk] = dm
                srcmap[kbase:kbase + nchunk] = sm
                dr = (dm % 128).astype(np.float32)
                dr[dm < 0] = 0
                dstrel[:, kbase:kbase + nchunk] = dr.T
                gidx_parts.append(_wrap_idx(idx))
                cnt16.append(_round16(ne))
        # host-built one-hot: [128, ktot, 128] bf16 bits; pad slots all-zero
        oh = np.zeros((128, ktot, 128), np.uint16)
        pp = np.arange(128)[:, None]
        kk_ = np.arange(ktot)[None, :]
        dr = dstrel.astype(np.int64)
        oh[pp, kk_, dr] = 0x3F80
        oh[(dstmap.T < 0)] = 0
        per_core.append(dict(
            gidx=np.ascontiguousarray(np.concatenate(gidx_parts, axis=1)),
            ohs=np.ascontiguousarray(oh.reshape(128, ktot * 128).view(np.int16)),
            srcmap=srcmap, dstmap=dstmap,
            gcnt=np.tile(np.asarray(cnt16, np.int32), (128, 1))))
    nidx16 = sum(r // 16 for cl in calls for _, r, _ in cl)
    ncalls = sum(len(cl) for cl in calls)
    return dict(calls=calls, kofs=kofs, K_b=K_b, KMAX=KMAX, ktot=ktot,
                nidx16=nidx16, ncalls=ncalls, per_core=per_core)


def _wext(W, a_s, a_d):
    """[128, 136] = [W | v_src | v_dst] zero-padded; v_* = W @ att_* per head
    so a_src/a_dst fall out of the same dense matmul as h."""
    W = np.asarray(W, np.float32)
    a_s = np.asarray(a_s, np.float32)
    a_d = np.asarray(a_d, np.float32)
    heads, ch = a_s.shape
    out = np.zeros((128, 136), np.float32)
    out[:W.shape[0], :W.shape[1]] = W
    for h in range(heads):
        out[:W.shape[0], 128 + h] = W[:, h * ch:(h + 1) * ch] @ a_s[h]
        out[:W.shape[0], 132 + h] = W[:, h * ch:(h + 1) * ch] @ a_d[h]
    return out


def _assemble(slabs, cols):
    t = np.zeros((TROWS, cols), np.float32)
    t[:NPAD] = np.concatenate(slabs, axis=0)
    t[N:] = 0.0
    return t


def _streams(aux, pc, nh):
    """per-edge a_src and a_dst dense streams [128, ktot*nh] f32 in
    (p, k, h) layout; pad slots get -1e30 / 0 so w becomes exactly 0."""
    out = []
    for m, col0, fill in ((pc["srcmap"], 0, -1e30), (pc["dstmap"], 4, 0.0)):
        val = np.full((m.shape[0], 128, nh), fill, np.float32)
        ok = m >= 0
        val[ok] = aux[m[ok]][:, col0:col0 + nh]
        out.append(np.ascontiguousarray(
            val.transpose(1, 0, 2).reshape(128, -1)))
    return out


# --------------------------------------------------------------------------
# device kernels
# --------------------------------------------------------------------------
_KER_CACHE = {}


def _get_kernels(meta):
    key = tuple(tuple(cl) for cl in meta["calls"])
    if key not in _KER_CACHE:
        _KER_CACHE[key] = _build_kernels(meta)
    return _KER_CACHE[key]


def _build_kernels(meta):
    import concourse.mybir as mybir
    import concourse.tile as tile
    from concourse import bacc

    calls, kofs, K_b, KMAX, ktot, nidx16, ncalls = (
        meta["calls"], meta["kofs"], meta["K_b"], meta["KMAX"],
        meta["ktot"], meta["nidx16"], meta["ncalls"])
    dt = mybir.dt
    AF = mybir.ActivationFunctionType

    def new_nc():
        return bacc.Bacc("TRN2", target_bir_lowering=False, debug=False,
                         num_swdge_queues=4)

    # ---- L0: dense only -------------------------------------------------
    nc0 = new_nc()
    xT = nc0.declare_dram_parameter("xT", [128, NPC], dt.int16, False)
    w0 = nc0.declare_dram_parameter("wext", [128, 136], dt.int16, False)
    slab0 = nc0.declare_dram_parameter("slab", [NPC, ROWF], dt.float32, True)
    aux0 = nc0.declare_dram_parameter("aux", [NPC, 8], dt.float32, True)
    with tile.TileContext(nc0) as tc:
        with tc.tile_pool(name="p", bufs=4) as pool, \
             tc.tile_pool(name="c", bufs=1) as cpool, \
             tc.tile_pool(name="ps", bufs=4, space="PSUM") as pps:
            xts = cpool.tile([128, NPC], dt.int16, tag="xt")
            nc0.sync.dma_start(out=xts[:], in_=xT[:])
            ws = cpool.tile([128, 136], dt.int16, tag="w")
            nc0.sync.dma_start(out=ws[:], in_=w0[:])
            wsb = ws[:].bitcast(dt.bfloat16)
            xtb = xts[:].bitcast(dt.bfloat16)
            # 3 blocks per PSUM tile to amortize per-iteration dispatch latency
            for b0i in range(0, BLOCKS, 3):
                nb = min(3, BLOCKS - b0i)
                ps = pps.tile([128, 3 * 136], dt.float32, tag="h")
                for j in range(nb):
                    nc0.tensor.matmul(
                        ps[:, j * 136:(j + 1) * 136],
                        lhsT=xtb[:, (b0i + j) * 128:(b0i + j + 1) * 128],
                        rhs=wsb[:], start=True, stop=True)
                psv = ps[:].rearrange("p (j m) -> p j m", m=136)
                row = pool.tile([128, 3 * ROWF], dt.float32, tag="row")
                rb = row[:].bitcast(dt.bfloat16)
                nc0.scalar.activation(
                    rb[:].rearrange("p (j m) -> p j m", m=128)[:, 0:nb, :],
                    psv[:, 0:nb, 0:128], AF.Copy)
                ax = pool.tile([128, 3 * 8], dt.float32, tag="aux")
                nc0.vector.tensor_copy(
                    ax[:].rearrange("p (j m) -> p j m", m=8)[:, 0:nb, :],
                    psv[:, 0:nb, 128:136])
                nc0.sync.dma_start(
                    out=slab0[b0i * 128:(b0i + nb) * 128, :]
                        .rearrange("(j p) w -> p j w", j=nb),
                    in_=row[:].rearrange("p (j w) -> p j w", w=ROWF)[:, 0:nb, :])
                nc0.sync.dma_start(
                    out=aux0[b0i * 128:(b0i + nb) * 128, :]
                        .rearrange("(j p) w -> p j w", j=nb),
                    in_=ax[:].rearrange("p (j w) -> p j w", w=8)[:, 0:nb, :])
    _split_multiwait_ctrl(nc0)
    nc0.compile()

    # ---- edge phase (+ optional fused next dense) -----------------------
    def build_edge(last):
        nc = new_nc()
        table = nc.declare_dram_parameter("table", [TROWS, ROWF], dt.float32, False)
        selftab = nc.declare_dram_parameter("selftab", [NPC, ROWF], dt.float32, False)
        gidx = nc.declare_dram_parameter("gidx", [128, nidx16], dt.int16, False)
        ohs = nc.declare_dram_parameter("ohs", [128, ktot * 128], dt.int16, False)
        NH = 1 if last else 4
        HC = 64 if last else 128
        MC = HC + NH
        C = HC // NH
        asrce = nc.declare_dram_parameter("asrce", [128, ktot * NH], dt.float32, False)
        adste = nc.declare_dram_parameter("adste", [128, ktot * NH], dt.float32, False)
        bias = nc.declare_dram_parameter("bias", [128, HC], dt.float32, False)
        if last:
            out = nc.declare_dram_parameter("out", [NPC, HC], dt.float32, True)
        else:
            ident = nc.declare_dram_parameter("ident", [128, 128], dt.float32, False)
            wnext = nc.declare_dram_parameter("wext", [128, 136], dt.int16, False)
            out = nc.declare_dram_parameter("slab", [NPC, ROWF], dt.float32, True)
            auxo = nc.declare_dram_parameter("aux", [NPC, 8], dt.float32, True)

        with tile.TileContext(nc) as tc:
            with tc.tile_pool(name="c", bufs=1) as cpool, \
                 tc.tile_pool(name="g", bufs=6) as gpool, \
                 tc.tile_pool(name="o", bufs=4) as opool, \
                 tc.tile_pool(name="w", bufs=4) as wpool, \
                 tc.tile_pool(name="ps", bufs=2, space="PSUM") as pps, \
                 tc.tile_pool(name="ps2", bufs=2, space="PSUM") as pps2:
                regs = {}
                for cl in calls:
                    for _, rows, _ in cl:
                        if rows not in regs:
                            regs[rows] = nc.gpsimd.to_reg(rows)
                # streams load in four chunks, FIRST chunk of every stream
                # before any remainder, so block 0's deps land early
                idxs = cpool.tile([128, nidx16], dt.int16, tag="gidx")
                asr = cpool.tile([128, ktot * NH], dt.float32, tag="asr")
                ads = cpool.tile([128, ktot * NH], dt.float32, tag="ads")
                specs = [(idxs, gidx, nidx16), (asr, asrce, ktot * NH),
                         (ads, adste, ktot * NH)]
                for phase in range(4):
                    for t, par, w in specs:
                        qc = _round16(w // 4 + 1)
                        o = phase * qc
                        e = min(o + qc, w)
                        if o < e:
                            nc.sync.dma_start(out=t[:, o:e], in_=par[:, o:e])
                bia = cpool.tile([128, HC], dt.float32, tag="bias")
                nc.sync.dma_start(out=bia[:], in_=bias[:])
                if not last:
                    idn = cpool.tile([128, 128], dt.float32, tag="ident")
                    nc.sync.dma_start(out=idn[:], in_=ident[:])
                    wnx = cpool.tile([128, 136], dt.int16, tag="wext")
                    nc.sync.dma_start(out=wnx[:], in_=wnext[:])
                    wnxb = wnx[:].bitcast(dt.bfloat16)

                # zero-fill the rotating gather buffers so never-written
                # slots hold finite bits (stale*0 must be 0, not NaN)
                for i in range(6):
                    gz = gpool.tile([128, KMAX, ROWF], dt.float32, tag="G")
                    nc.vector.memset(gz[:], 0.0)

                tab_lo = table[0:SPLIT, :]
                tab_hi = table[SPLIT:TROWS, :]
                ioff = 0
                qn = 0
                for b in range(BLOCKS):
                    K = K_b[b]
                    koff = kofs[b]
                    G = gpool.tile([128, KMAX, ROWF], dt.float32, tag="G")
                    nc.sync.dma_start(out=G[:, 0:1, :],
                                      in_=selftab[b * 128:(b + 1) * 128, :])
                    for hf, rows, kk in calls[b]:
                        nch = -(-rows // 128)
                        nc.gpsimd.dma_gather(
                            G[:, kk:kk + nch, :],
                            tab_lo if hf == 0 else tab_hi,
                            idxs[:, ioff:ioff + rows // 16],
                            num_idxs=rows, num_idxs_reg=regs[rows],
                            elem_size=ROWF, queue_num=qn)
                        qn = (qn + 1) % 4
                        ioff += rows // 16
                    Gb = G[:].bitcast(dt.bfloat16)   # [128, KMAX, 128]

                    # w = exp(lrelu(a_src + a_dst)); pad slots -> exactly 0
                    wv = wpool.tile([128, KMAX * NH], dt.float32, tag="wv")
                    nc.vector.tensor_add(
                        wv[:, 0:K * NH], asr[:, koff * NH:(koff + K) * NH],
                        ads[:, koff * NH:(koff + K) * NH])
                    nc.scalar.activation(wv[:, 0:K * NH], wv[:, 0:K * NH],
                                         AF.Prelu, alpha=NEG)
                    nc.scalar.activation(wv[:, 0:K * NH], wv[:, 0:K * NH],
                                         AF.Exp)
                    wb = wpool.tile([128, KMAX * NH], dt.bfloat16, tag="wbf")
                    nc.scalar.activation(wb[:, 0:K * NH], wv[:, 0:K * NH],
                                         AF.Copy)

                    # one-hot(dst_rel) [128, K, 128] bf16: host-built, DMA'd
                    ohi = opool.tile([128, KMAX * 128], dt.int16, tag="oh")
                    nc.sync.dma_start(out=ohi[:, 0:K * 128],
                                      in_=ohs[:, koff * 128:(koff + K) * 128])
                    oh = ohi[:].bitcast(dt.bfloat16)

                    # M = [h*w | w] bf16
                    M = wpool.tile([128, KMAX * MC], dt.bfloat16, tag="M")
                    Mv = M[:, 0:K * MC].rearrange("p (k m) -> p k m", m=MC)
                    nc.vector.tensor_mul(
                        Mv[:, :, 0:HC].rearrange("p k (h c) -> p k h c", c=C),
                        Gb[:, 0:K, 0:HC].rearrange("p k (h c) -> p k h c", c=C),
                        wb[:, 0:K * NH].rearrange("p (k h o) -> p k h o", h=NH, o=1)
                            .to_broadcast([128, K, NH, C]))
                    nc.scalar.activation(
                        Mv[:, :, HC:MC],
                        wb[:, 0:K * NH].rearrange("p (k h) -> p k h", h=NH),
                        AF.Copy)

                    T = pps.tile([128, MC], dt.float32, tag="T")
                    for k in range(K):
                        nc.tensor.matmul(T[:],
                                         lhsT=oh[:, k * 128:(k + 1) * 128],
                                         rhs=Mv[:, k, :],
                                         start=(k == 0), stop=(k == K - 1))

                    rcp = wpool.tile([128, NH], dt.float32, tag="rcp")
                    nc.vector.reciprocal(rcp[:], T[:, HC:MC])
                    xp = wpool.tile([128, HC], dt.float32, tag="xp")
                    nc.vector.tensor_mul(
                        xp[:].rearrange("p (h c) -> p h c", c=C),
                        T[:, 0:HC].rearrange("p (h c) -> p h c", c=C),
                        rcp[:].rearrange("p (h o) -> p h o", o=1)
                            .to_broadcast([128, NH, C]))
                    nc.vector.tensor_add(xp[:], xp[:], bia[:])
                    nc.scalar.activation(xp[:], xp[:], AF.Prelu, alpha=NEG)
                    if last:
                        nc.sync.dma_start(out=out[b * 128:(b + 1) * 128, :],
                                          in_=xp[:])
                    else:
                        pt = pps2.tile([128, 128], dt.float32, tag="xt")
                        nc.tensor.transpose(out=pt[:], in_=xp[:],
                                            identity=idn[:])
                        xt = wpool.tile([128, 128], dt.bfloat16, tag="xts")
                        nc.scalar.activation(xt[:], pt[:], AF.Copy)
                        ph = pps2.tile([128, 136], dt.float32, tag="h2")
                        nc.tensor.matmul(ph[:], lhsT=xt[:], rhs=wnxb[:],
                                         start=True, stop=True)
                        row = wpool.tile([128, ROWF], dt.float32, tag="row")
                        rb = row[:].bitcast(dt.bfloat16)
                        nc.scalar.activation(rb[:], ph[:, 0:128], AF.Copy)
                        ax = wpool.tile([128, 8], dt.float32, tag="aux")
                        nc.scalar.activation(ax[:], ph[:, 128:136], AF.Copy)
                        nc.sync.dma_start(out=out[b * 128:(b + 1) * 128, :],
                                          in_=row[:])
                        nc.sync.dma_start(out=auxo[b * 128:(b + 1) * 128, :],
                                          in_=ax[:])
        _split_multiwait_ctrl(nc)
        nc.compile()
        return nc

    return nc0, build_edge(False), build_edge(True)


# --------------------------------------------------------------------------
# entry point
# --------------------------------------------------------------------------
def kernel(x, edge_index, W0, as0, ad0, b0, W1, as1, ad1, b1, W2, as2, ad2, b2):
    _install_ntff_hook()
    from concourse.bass_utils import run_bass_kernel_spmd

    x = np.asarray(x, np.float32)
    meta = _prep_graph(np.asarray(edge_index))
    nc0, nc12, nc3 = _get_kernels(meta)
    cores = list(range(NCORES))
    trace = bool(os.environ.get("BASS_TRACE"))

    import ml_dtypes
    bf = lambda a: np.ascontiguousarray(
        np.ascontiguousarray(a).astype(ml_dtypes.bfloat16).view(np.int16))
    iota = bf(np.tile(np.arange(128, dtype=np.float32), (128, 1)))
    ident = np.eye(128, dtype=np.float32)
    w0e, w1e, w2e = (bf(_wext(W0, as0, ad0)), bf(_wext(W1, as1, ad1)),
                     bf(_wext(W2, as2, ad2)))

    total_ns = [0]

    def run(nc, maps):
        last = None
        for attempt in range(3):
            try:
                r = run_bass_kernel_spmd(nc, maps, core_ids=cores, trace=trace)
                if r.exec_time_ns:
                    total_ns[0] += int(r.exec_time_ns)
                    if os.environ.get("KERNEL_VERBOSE"):
                        print(f"[launch] exec={r.exec_time_ns}ns", file=sys.stderr)
                return r.results
            except Exception as e:  # intermittent NRT exec-unit crashes
                last = e
        raise last

    xT = np.zeros((128, NPAD), np.float32)
    xT[:, :N] = x.T
    xTb = bf(xT)
    res = run(nc0, [{"xT": np.ascontiguousarray(xTb[:, c * NPC:(c + 1) * NPC]),
                     "wext": w0e} for c in cores])
    table = _assemble([res[c]["slab"] for c in cores], ROWF)
    aux = _assemble([res[c]["aux"] for c in cores], 8)

    def edge_maps(tab, aux, wnext, bias_vec, hc, nh):
        bias = np.tile(np.asarray(bias_vec, np.float32)[:hc], (128, 1))
        maps = []
        for c in cores:
            pc = meta["per_core"][c]
            asrce, adste = _streams(aux, pc, nh)
            m = {"table": tab, "gidx": pc["gidx"], "ohs": pc["ohs"],
                 "selftab": np.ascontiguousarray(tab[c * NPC:(c + 1) * NPC]),
                 "asrce": asrce, "adste": adste, "bias": bias}
            if wnext is not None:
                m["ident"] = ident
                m["wext"] = wnext
            maps.append(m)
        return maps

    res = run(nc12, edge_maps(table, aux, w1e, b0, 128, 4))
    table = _assemble([res[c]["slab"] for c in cores], ROWF)
    aux = _assemble([res[c]["aux"] for c in cores], 8)
    res = run(nc12, edge_maps(table, aux, w2e, b1, 128, 4))
    table = _assemble([res[c]["slab"] for c in cores], ROWF)
    aux = _assemble([res[c]["aux"] for c in cores], 8)
    res = run(nc3, edge_maps(table, aux, None, b2, 64, 1))
    out = np.concatenate([res[c]["out"] for c in cores], axis=0)[:N]
    kernel.last_exec_ns = total_ns[0]
    return np.ascontiguousarray(out, dtype=np.float32)
